# revision 3
# baseline (speedup 1.0000x reference)
"""Trainium (Bass/Tile) kernel for nn_DiceLoss: 8-core row-block-sharded
dice loss over a 4096x4096 segmented image.

loss = 1 - mean_c( 2*A_c / (B_c + C_c + 1e-10) ) with, per class c:
  A_c = #pixels(pred[seg]==c and tgt==c)
  B_c = #pixels(pred[seg]==c)
  C_c = #pixels(tgt==c)
where pred = argmax(output, axis=1) (first-max), seg/tgt are the (N,N)
int index images.

Device strategy (per core, 512 image rows = 2M pixels as [128, 16384]):
joint-histogram via TensorE outer-products — no per-pixel gather.

  - host packs key = seg*8 + tgt into int16 (lossless bit-pack; 4x
    less HBM/upload traffic than the raw int32/int64 pair)
  - argmax(logits) -> pred as predv[128, 16] (predv[q, r] = pred[16q+r])
  - hi = key >> 7 (= seg >> 4), lo = key & 127 (= (seg & 15)*8 + tgt)
  - for each column of 128 pixels: build 128-bin one-hots of hi and lo
    in a bins-major [P, 128, FI] layout (keeps every DVE operand
    2-byte innermost-packed -> 2x mode) and accumulate
    psum[hi_bin, lo_bin] += OH_hi^T @ OH_lo on TensorE (bf16)
  - PSUM [128, 128] is then the exact joint histogram H[q, r*8+t] of
    (seg, tgt); contract with pred masks for the 24 per-class counts
  - one f32 matmul against ones reduces across partitions; 24 floats
    DMA out; host sums across the 8 cores (the C-length "all-reduce")
    and applies the exact fp32 dice formula
"""

import os

import numpy as np

import concourse.bacc as bacc
import concourse.mybir as mybir
import concourse.tile as tile
from concourse.bass_utils import run_bass_kernel_spmd

P = 128
V = 2048     # vertices (rows of `output`)
C = 8        # classes
N = 4096     # image side
NCORES = 8
ROWS_PER_CORE = N // NCORES          # 512
FREE = ROWS_PER_CORE * N // P        # 16384 pixels per partition
BINS = 128
FI = 64                              # columns per one-hot build
NSUB = FREE // FI                    # 256

_PROGRAM_CACHE = {}
LAST_RESULTS = None


def _build_program():
    f32 = mybir.dt.float32
    bf16 = mybir.dt.bfloat16
    i16 = mybir.dt.int16

    nc = bacc.Bacc("TRN2", target_bir_lowering=False, debug=False,
                   num_devices=NCORES)
    logits_ap = nc.dram_tensor("logits", [P, 128], f32, kind="ExternalInput")
    key_ap = nc.dram_tensor("key16", [P, FREE], i16, kind="ExternalInput")
    counts_ap = nc.dram_tensor("counts", [24], f32, kind="ExternalOutput")

    iota_np = np.tile(np.arange(BINS, dtype=np.int16).reshape(1, BINS, 1),
                      (P, 1, 1))
    iota_d = nc.inline_tensor(iota_np, name="iota_bins")

    with tile.TileContext(nc) as tc:
        with (
            tc.tile_pool(name="const", bufs=1) as pool_c,
            tc.tile_pool(name="big", bufs=1) as pool_b,
            tc.tile_pool(name="oh", bufs=2) as pool_oh,
            tc.tile_pool(name="epi", bufs=1) as pool_e,
            tc.tile_pool(name="psum", bufs=1, space="PSUM") as pool_psum,
        ):
            # ---- constants ----
            iota_col = pool_c.tile([P, BINS, 1], i16, tag="iota_col")
            nc.sync.dma_start(out=iota_col[:, :, :], in_=iota_d[:, :, :])
            iotaT = pool_c.tile([P, BINS, FI], i16, tag="iotaT")
            nc.vector.tensor_copy(
                iotaT[:, :, :],
                iota_col[:, :, :].broadcast_to([P, BINS, FI]))

            # ---- phase A: pred = argmax(logits, axis=1), first-max ----
            ovt = pool_e.tile([P, 16, C], f32, tag="ovt")
            nc.sync.dma_start(out=ovt[:, :, :], in_=logits_ap[:, :])
            mx = pool_e.tile([P, 16], f32, tag="mx")
            nc.vector.tensor_reduce(mx[:, :], ovt[:, :, :],
                                    axis=mybir.AxisListType.X,
                                    op=mybir.AluOpType.max)
            predv = pool_e.tile([P, 16], f32, tag="predv")
            nc.vector.memset(predv[:, :], float(C - 1))
            eqm = pool_e.tile([P, 16], mybir.dt.uint8, tag="eqm")
            ctile = pool_e.tile([P, 16], f32, tag="ctile")
            for c in range(C - 2, -1, -1):
                nc.vector.tensor_tensor(eqm[:, :], ovt[:, :, c], mx[:, :],
                                        mybir.AluOpType.is_equal)
                nc.vector.memset(ctile[:, :], float(c))
                nc.vector.copy_predicated(predv[:, :], eqm[:, :], ctile[:, :])

            # ---- phase B: load key, extract hi/lo, histogram ----
            key = pool_b.tile([P, FREE], i16, tag="key")
            nc.sync.dma_start(out=key[:, :], in_=key_ap[:, :])
            hi = pool_b.tile([P, FREE], i16, tag="hi")
            lo = pool_b.tile([P, FREE], i16, tag="lo")
            nc.vector.tensor_scalar(out=hi[:, :], in0=key[:, :],
                                    scalar1=7, scalar2=None,
                                    op0=mybir.AluOpType.logical_shift_right)
            nc.vector.tensor_scalar(out=lo[:, :], in0=key[:, :],
                                    scalar1=127, scalar2=None,
                                    op0=mybir.AluOpType.bitwise_and)

            psumH = pool_psum.tile([BINS, BINS], f32, tag="psumH")
            for s in range(NSUB):
                jsl = slice(s * FI, (s + 1) * FI)
                ohh = pool_oh.tile([P, BINS, FI], bf16, tag="ohh")
                ohl = pool_oh.tile([P, BINS, FI], bf16, tag="ohl")
                nc.vector.tensor_tensor(
                    ohh[:, :, :], iotaT[:, :, :],
                    hi[:, jsl].unsqueeze(1).broadcast_to([P, BINS, FI]),
                    mybir.AluOpType.is_equal)
                nc.vector.tensor_tensor(
                    ohl[:, :, :], iotaT[:, :, :],
                    lo[:, jsl].unsqueeze(1).broadcast_to([P, BINS, FI]),
                    mybir.AluOpType.is_equal)
                for j in range(FI):
                    nc.tensor.matmul(psumH[:, :], ohh[:, :, j], ohl[:, :, j],
                                     start=(s == 0 and j == 0),
                                     stop=(s == NSUB - 1 and j == FI - 1),
                                     skip_group_check=True)

            # ---- phase C: contract H with pred masks -> 24 counts ----
            H = pool_e.tile([P, 16, C], f32, tag="H")
            nc.vector.tensor_copy(
                H[:, :, :].rearrange("p a b -> p (a b)"), psumH[:, :])
            rowsum = pool_e.tile([P, 16], f32, tag="rowsum")
            nc.vector.tensor_reduce(rowsum[:, :], H[:, :, :],
                                    axis=mybir.AxisListType.X,
                                    op=mybir.AluOpType.add)
            cnt = pool_e.tile([P, 24], f32, tag="cnt")
            mc = pool_e.tile([P, 16], f32, tag="mc")
            tmp = pool_e.tile([P, 16], f32, tag="tmp")
            for c in range(C):
                nc.vector.tensor_scalar(out=mc[:, :], in0=predv[:, :],
                                        scalar1=float(c), scalar2=None,
                                        op0=mybir.AluOpType.is_equal)
                nc.vector.tensor_tensor(tmp[:, :], mc[:, :], rowsum[:, :],
                                        mybir.AluOpType.mult)
                nc.vector.tensor_reduce(cnt[:, c:c + 1], tmp[:, :],
                                        axis=mybir.AxisListType.X,
                                        op=mybir.AluOpType.add)
                nc.vector.tensor_tensor(tmp[:, :], mc[:, :], H[:, :, c],
                                        mybir.AluOpType.mult)
                nc.vector.tensor_reduce(cnt[:, C + c:C + c + 1], tmp[:, :],
                                        axis=mybir.AxisListType.X,
                                        op=mybir.AluOpType.add)
                nc.vector.tensor_reduce(cnt[:, 16 + c:16 + c + 1], H[:, :, c],
                                        axis=mybir.AxisListType.X,
                                        op=mybir.AluOpType.add)

            ones_col = pool_e.tile([P, 1], f32, tag="ones_col")
            nc.vector.memset(ones_col[:, :], 1.0)
            psum24 = pool_psum.tile([24, 1], f32, tag="psum24")
            nc.tensor.matmul(psum24[:, :], cnt[:, :], ones_col[:, :],
                             start=True, stop=True, skip_group_check=True)
            out24 = pool_e.tile([24, 1], f32, tag="out24")
            nc.vector.tensor_copy(out24[:, :], psum24[:, :])
            nc.sync.dma_start(out=counts_ap[0:24], in_=out24[:, :])

    nc.compile()
    return nc


def _pack_key16(target, segments):
    """Lossless bit-pack of the two index images into one int16 word per
    pixel: bits 3..13 = segment id (< 2048), bits 0..2 = target class."""
    seg16 = segments.astype(np.int16)
    tgt16 = target.astype(np.int16)
    return (seg16 << np.int16(3)) | tgt16


def _in_maps_for(output, target, segments):
    logits = np.ascontiguousarray(output, dtype=np.float32).reshape(P, 128)
    key16 = _pack_key16(target, segments)
    in_maps = []
    for core in range(NCORES):
        r0, r1 = core * ROWS_PER_CORE, (core + 1) * ROWS_PER_CORE
        in_maps.append({"logits": logits,
                        "key16": key16[r0:r1].reshape(P, FREE)})
    return in_maps


def kernel(output, target, segments):
    global LAST_RESULTS
    output = np.asarray(output)
    target = np.asarray(target)
    segments = np.asarray(segments)
    assert output.shape == (V, C)
    assert target.shape == (N, N) and segments.shape == (N, N)

    if "p" not in _PROGRAM_CACHE:
        _PROGRAM_CACHE["p"] = _build_program()
    nc = _PROGRAM_CACHE["p"]

    in_maps = _in_maps_for(output, target, segments)
    trace = bool(int(os.environ.get("DICE_TRACE", "0")))
    res = run_bass_kernel_spmd(nc, in_maps, core_ids=list(range(NCORES)),
                               trace=trace)
    LAST_RESULTS = res

    tot = np.zeros(24, dtype=np.float64)
    for core in range(NCORES):
        tot += res.results[core]["counts"].astype(np.float64)
    B = tot[0:8].astype(np.float32)
    A = tot[8:16].astype(np.float32)
    Cc = tot[16:24].astype(np.float32)

    intersection = np.float32(2.0) * A
    union = B + Cc
    score = intersection / (union + np.float32(1e-10))
    return np.float32(1.0) - np.float32(score.mean(dtype=np.float32))


def _make_runner(nc, in_maps):
    """Steady-state runner for a compiled program: jit once, keep inputs
    device-resident, time repeated executes."""
    import time

    import jax
    from jax.sharding import Mesh, PartitionSpec
    from jax.experimental.shard_map import shard_map

    from concourse import bass2jax

    bass2jax.install_neuronx_cc_hook()
    part_name = (nc.partition_id_tensor.name if nc.partition_id_tensor
                 else None)
    in_names, out_names, out_avals, zero_outs = [], [], [], []
    for alloc in nc.m.functions[0].allocations:
        if not isinstance(alloc, mybir.MemoryLocationSet):
            continue
        name = alloc.memorylocations[0].name
        if alloc.kind == "ExternalInput":
            if name != part_name:
                in_names.append(name)
        elif alloc.kind == "ExternalOutput":
            out_names.append(name)
            shape = tuple(alloc.tensor_shape)
            dtype = mybir.dt.np(alloc.dtype)
            out_avals.append(jax.core.ShapedArray(shape, dtype))
            zero_outs.append(np.zeros(shape, dtype))
    n_params, n_outs = len(in_names), len(out_avals)
    all_names = in_names + out_names + ([part_name] if part_name else [])

    def _body(*args):
        operands = list(args)
        if part_name is not None:
            operands.append(bass2jax.partition_id_tensor())
        return tuple(bass2jax._bass_exec_p.bind(
            *operands, out_avals=tuple(out_avals), in_names=tuple(all_names),
            out_names=tuple(out_names), lowering_input_output_aliases=(),
            sim_require_finite=True, sim_require_nnan=True, nc=nc))

    devices = jax.devices()[:NCORES]
    mesh = Mesh(np.asarray(devices), ("core",))
    sharded = jax.jit(
        shard_map(_body, mesh=mesh,
                  in_specs=(PartitionSpec("core"),) * (n_params + n_outs),
                  out_specs=(PartitionSpec("core"),) * n_outs,
                  check_rep=False),
        donate_argnums=tuple(range(n_params, n_params + n_outs)),
        keep_unused=True)
    dev_in = [jax.device_put(np.concatenate(
        [np.asarray(m[nm]) for m in in_maps], axis=0)) for nm in in_names]
    for a in dev_in:
        a.block_until_ready()

    def zeros():
        return [np.zeros((NCORES * z.shape[0], *z.shape[1:]), z.dtype)
                for z in zero_outs]

    jax.block_until_ready(sharded(*dev_in, *zeros()))

    def run_once():
        z = zeros()
        t0 = time.perf_counter()
        jax.block_until_ready(sharded(*dev_in, *z))
        return (time.perf_counter() - t0) * 1e9

    return run_once


def measure_exec_ns(inputs, reps=12):
    """Estimate on-device kernel time: steady-state wall delta between the
    dice NEFF and a size-matched trivial NEFF (same declared inputs, no
    compute), interleaved to cancel axon-tunnel drift. Matching the input
    footprint cancels the per-execute input-shipping overhead of the axon
    path, which is not device execution time."""
    import concourse.tile as tile_mod

    nc = _PROGRAM_CACHE["p"]
    in_maps = _in_maps_for(np.asarray(inputs["output"]),
                           np.asarray(inputs["target"]),
                           np.asarray(inputs["segments"]))
    run_dice = _make_runner(nc, in_maps)

    hnc = bacc.Bacc("TRN2", target_bir_lowering=False, debug=False,
                    num_devices=NCORES)
    hl = hnc.dram_tensor("logits", [P, 128], mybir.dt.float32,
                         kind="ExternalInput")
    hk = hnc.dram_tensor("key16", [P, FREE], mybir.dt.int16,
                         kind="ExternalInput")
    y = hnc.dram_tensor("counts", [24], mybir.dt.float32,
                        kind="ExternalOutput")
    with tile_mod.TileContext(hnc) as tc:
        with tc.tile_pool(name="p", bufs=2) as pool:
            t = pool.tile([128, 512], mybir.dt.int16)
            hnc.sync.dma_start(out=t[:, :], in_=hk[:, 0:512])
            tf = pool.tile([128, 128], mybir.dt.float32)
            hnc.sync.dma_start(out=tf[:, :], in_=hl[:, :])
            hnc.vector.tensor_scalar_mul(tf[:, :], tf[:, :], 2.0)
            hnc.sync.dma_start(out=y[:], in_=tf[0:24, 0:1])
    hnc.compile()
    run_hello = _make_runner(hnc, in_maps)

    dice, hello = [], []
    for _ in range(reps):
        hello.append(run_hello())
        dice.append(run_dice())
    return float(np.median(np.array(dice)) - np.median(np.array(hello)))


if __name__ == "__main__":
    rng = np.random.default_rng(0)
    out = rng.standard_normal((V, C)).astype(np.float32)
    tgt = rng.integers(0, C, size=(N, N)).astype(np.int32)
    seg = rng.integers(0, V, size=(N, N)).astype(np.int32)
    print("loss:", kernel(output=out, target=tgt, segments=seg))
